# revision 2
# baseline (speedup 1.0000x reference)
"""Trainium2 Bass kernel for nn_DecoderSmoothedMaxPoolingLoss — v4 (f16 bulk).

Structure (see v2/v3): valid-only streaming of X[b, :len_b, :], row-balanced
across 8 cores, window terms from host-gathered (8,60) f32 windows, fused
ACT Ln+accum, final ones-matmul collapse, host sums 8 partial scalars.

v4 specifics:
  - Bulk ships as float16 (RN cast; values that round to 1.0 are clamped
    to 1-2^-11).  Error vs f32 reference ~0.02% — f16's 10 mantissa bits
    keep ln(1-x) accurate much closer to x=1 than bf16's 7.
  - Per chunk the DVE computes u' = 64-64x in place (4x rate) and pairs
    halves p' = u'_lo * u'_hi (2x rate).  All values stay in f16 normal
    range (p' in [2^-10, 4096]) — no subnormal/FTZ hazard.  ACT then does
    Ln(p' * 2^-12) via its free input affine (fp32 internally), with
    fused accum -> sum ln(u1*u2) per partition.  Zero-padding gives
    p'=4096 -> Ln(1)=0 exactly.
  - Fewer, larger DMAs (3x1.2MB + a tapering tail) on the sync HWDGE
    ring: per-DMA issue cost (~0.8us) stops eating the 13.4us stream,
    and the small tail chunks shorten the post-stream pipeline drain.
  - aux loads ride SWDGE (gpsimd) so the sync ring starts on bulk
    immediately; the window path's final Ln is emitted after the chunk
    ACTs so the scalar queue never stalls them.
"""

import numpy as np

import concourse.bass as bass
import concourse.tile as tile
from concourse import bacc
from concourse import mybir
from concourse import bass_utils

AF = mybir.ActivationFunctionType
ALU = mybir.AluOpType
AX = mybir.AxisListType
FP = mybir.dt.float32
F16 = mybir.dt.float16

B, T, K = 64, 4000, 100
WIN, OFFSET_D, TRUNC, SIGMA = 60, 40, 21, 9
EPS = 1e-8
NCORES = 8
BLOC = B // NCORES
AW = 2 * WIN + BLOC
F16_BELOW_ONE = np.float16(1.0 - 2.0 ** -11)


def _chunk_widths(ftot):
    """Split ftot columns into DMA chunks: 3 big + tapering tail, all
    multiples of 4."""
    big = (ftot // 4) & ~3
    rem = ftot - 3 * big
    h1 = (rem // 2) & ~3
    q = (rem - h1) // 2 & ~3
    last = rem - h1 - q
    ws = [big, big, big, h1, q, last]
    assert sum(ws) == ftot and all(w > 0 and w % 4 == 0 for w in ws), ws
    return ws


def _filt_np():
    half = TRUNC // 2
    x = np.arange(-half, half + 1, dtype=np.float32)
    g = np.exp(-0.5 * (x / SIGMA) ** 2).astype(np.float32)
    g = g / g.sum()
    f = np.zeros(WIN, np.float32)
    c = WIN // 2
    f[c - half:c + half + 1] = g
    return f


def _conv_matrix():
    f = _filt_np()
    pl = (WIN - 1) // 2
    idx = np.arange(WIN)
    u = idx[:, None] - idx[None, :] + pl
    M = np.where((u >= 0) & (u < WIN), f[np.clip(u, 0, WIN - 1)], 0.0)
    return M.astype(np.float32)


_NC_CACHE = {}


def _build_program(ftot):
    if ftot in _NC_CACHE:
        return _NC_CACHE[ftot]
    ws = _chunk_widths(ftot)
    nch = len(ws)

    nc = bacc.Bacc("TRN2", debug=False)
    Vd = nc.dram_tensor("V", [128 * ftot], F16, kind="ExternalInput").ap()
    auxW = nc.dram_tensor("auxW", [BLOC, AW], FP, kind="ExternalInput").ap()
    auxM = nc.dram_tensor("auxM", [WIN, WIN], FP, kind="ExternalInput").ap()
    outd = nc.dram_tensor("out", [1, 1], FP, kind="ExternalOutput").ap()

    with tile.TileContext(nc) as tc:
        with tc.tile_pool(name="xin", bufs=1) as xin_pool, \
             tc.tile_pool(name="pair", bufs=1) as pair_pool, \
             tc.tile_pool(name="small", bufs=1) as small, \
             tc.tile_pool(name="psum", bufs=1, space="PSUM") as psum:

            # aux loads first on the sync HWDGE ring (land before chunk 0)
            auxW_sb = small.tile([BLOC, AW], FP)
            nc.sync.dma_start(out=auxW_sb[:], in_=auxW)
            auxM_sb = small.tile([WIN, WIN], FP)
            nc.sync.dma_start(out=auxM_sb[:], in_=auxM)

            win_sl = auxW_sb[0:BLOC, 0:WIN]
            valid_sl = auxW_sb[0:BLOC, WIN:2 * WIN]
            I8_sl = auxW_sb[0:BLOC, 2 * WIN:AW]

            Call = small.tile([128, nch + 2], FP)
            nc.vector.memset(Call[:], 0.0)

            # bulk DMAs: chunk i is a contiguous 128*w block of the slab
            xtiles = []
            off = 0
            for i, w in enumerate(ws):
                xb = xin_pool.tile([128, w], F16, tag=f"xb{i}")
                nc.sync.dma_start(
                    out=xb[:],
                    in_=Vd[off:off + 128 * w].rearrange("(p f) -> p f", p=128))
                xtiles.append(xb)
                off += 128 * w

            # lnw early: warms the Ln table during the stream and is the
            # only scalar op ahead of the chunk ACTs
            lnw = small.tile([BLOC, WIN], FP)
            nc.scalar.activation(out=lnw[:], in_=win_sl, func=AF.Ln,
                                 bias=1.0, scale=-1.0)

            # bulk: u' = 64-64x (DVE 4x), p' = u'_lo*u'_hi (DVE 2x),
            # ACT Ln(p'/4096) + accum
            for i, w in enumerate(ws):
                xb = xtiles[i]
                h = w // 2
                nc.vector.tensor_scalar(out=xb[:], in0=xb[:],
                                        scalar1=-64.0, scalar2=64.0,
                                        op0=ALU.mult, op1=ALU.add)
                pr = pair_pool.tile([128, h], F16, tag=f"pr{i}")
                nc.vector.tensor_tensor(out=pr[:], in0=xb[:, 0:h],
                                        in1=xb[:, h:w], op=ALU.mult)
                nc.scalar.activation(out=pr[:], in_=pr[:], func=AF.Ln,
                                     scale=2.0 ** -12,
                                     accum_out=Call[:, i:i + 1])

            # window DVE/PE chain after the bulk DVE ops (the stream keeps
            # DVE saturated; this short chain runs in the drain)
            lnwv = small.tile([BLOC, WIN], FP)
            nc.vector.tensor_tensor(out=lnwv[:], in0=lnw[:], in1=valid_sl,
                                    op=ALU.mult)
            ecol = small.tile([BLOC, 1], FP)
            nc.vector.tensor_reduce(out=ecol[:], in_=lnwv[:], axis=AX.X,
                                    op=ALU.add)
            nc.vector.tensor_scalar_mul(Call[0:BLOC, nch:nch + 1], ecol[:],
                                        -1.0)
            winv = small.tile([BLOC, WIN], FP)
            nc.vector.tensor_tensor(out=winv[:], in0=win_sl, in1=valid_sl,
                                    op=ALU.mult)
            wvt_ps = psum.tile([WIN, BLOC], FP)
            nc.tensor.matmul(out=wvt_ps[:], lhsT=winv[:], rhs=I8_sl,
                             start=True, stop=True)
            wvt = small.tile([WIN, BLOC], FP)
            nc.vector.tensor_copy(out=wvt[:], in_=wvt_ps[:])
            sm_ps = psum.tile([BLOC, WIN], FP)
            nc.tensor.matmul(out=sm_ps[:], lhsT=wvt[:], rhs=auxM_sb[:],
                             start=True, stop=True)
            smc = small.tile([BLOC, WIN], FP)
            nc.vector.tensor_scalar(out=smc[:], in0=sm_ps[:],
                                    scalar1=EPS, scalar2=1.0,
                                    op0=ALU.max, op1=ALU.min)
            smv = small.tile([BLOC, WIN], FP)
            nc.vector.tensor_tensor(out=smv[:], in0=smc[:], in1=valid_sl,
                                    op=ALU.mult)
            mx = small.tile([BLOC, 1], FP)
            nc.vector.tensor_reduce(out=mx[:], in_=smv[:], axis=AX.X,
                                    op=ALU.max)

            # window final Ln after the chunk ACTs
            nc.scalar.activation(out=Call[0:BLOC, nch + 1:nch + 2],
                                 in_=mx[:], func=AF.Ln)

            # final: loss = -(sum of all columns)
            ones = small.tile([128, 1], FP)
            nc.vector.memset(ones[:], 1.0)
            tot_ps = psum.tile([1, nch + 2], FP)
            nc.tensor.matmul(out=tot_ps[:], lhsT=ones[:], rhs=Call[:],
                             start=True, stop=True)
            tot = small.tile([1, 1], FP)
            nc.vector.tensor_reduce(out=tot[:], in_=tot_ps[:], axis=AX.X,
                                    op=ALU.add, negate=True)
            nc.sync.dma_start(out=outd, in_=tot[:])

    nc.compile()
    _NC_CACHE[ftot] = nc
    return nc


def _make_in_maps(X, lengths, tgt, w_end):
    X = np.ascontiguousarray(np.asarray(X, dtype=np.float32))
    lengths = np.asarray(lengths, dtype=np.int64)
    tgt = np.asarray(tgt, dtype=np.int64)
    w_end = np.asarray(w_end, dtype=np.int64)

    tau_s = np.maximum(0, w_end + OFFSET_D - WIN)
    tau_e = np.minimum(tau_s + WIN, lengths)
    Lw = tau_e - tau_s

    # RN cast to f16; x that rounds to 1.0 clamps to 1-2^-11
    Xh = X.astype(np.float16)
    np.minimum(Xh, F16_BELOW_ONE, out=Xh)

    R = int(lengths.sum())
    ftot = -(-R * K // (NCORES * 128))
    ftot += (-ftot) % 16
    epc = 128 * ftot
    Vall = np.zeros(NCORES * epc, np.float16)
    pos = 0
    for b in range(B):
        n = int(lengths[b]) * K
        Vall[pos:pos + n] = Xh[b, :lengths[b], :].reshape(-1)
        pos += n

    idx = tau_s[:, None] + np.arange(WIN)[None, :]
    win = X[np.arange(B)[:, None], idx, tgt[:, None]]
    valid = (np.arange(WIN)[None, :] < Lw[:, None]).astype(np.float32)
    I8 = np.eye(BLOC, dtype=np.float32)
    Mmat = _conv_matrix()

    in_maps = []
    for cr in range(NCORES):
        bs = slice(cr * BLOC, (cr + 1) * BLOC)
        auxW = np.concatenate([win[bs].astype(np.float32), valid[bs], I8],
                              axis=1)
        in_maps.append({
            "V": Vall[cr * epc:(cr + 1) * epc],
            "auxW": np.ascontiguousarray(auxW),
            "auxM": Mmat,
        })
    return ftot, in_maps


def kernel(X, lengths, tgt, w_end):
    ftot, in_maps = _make_in_maps(X, lengths, tgt, w_end)
    nc = _build_program(ftot)
    res = bass_utils.run_bass_kernel_spmd(
        nc, in_maps, core_ids=list(range(NCORES)))
    total = np.float32(0.0)
    for c in range(NCORES):
        total += np.float32(res.results[c]["out"][0, 0])
    return np.array(total, dtype=np.float32)


# revision 3
# speedup vs baseline: 1.0655x; 1.0655x over previous
"""Trainium2 Bass kernel for nn_DecoderSmoothedMaxPoolingLoss — v5.

Same math as v4 (f16 RN+clamp bulk, u'=64-64x / p'=u'lo*u'hi / Ln(p'/4096)
fused-accum; f32 window path; valid-only row-balanced streaming).

v5 scheduling changes:
  - One packed aux DMA on the scalar HWDGE ring (sync ring starts bulk
    immediately; aux lands early for the window path + Ln table warm).
  - Chunk widths taper down (big chunks early, small late): the ~2.2us
    DMA completion-receipt latency plus the per-chunk compute chain then
    overlaps the stream instead of dangling off its end.
  - Per-chunk mode: 'P' = DVE pair + half-size ACT, 'D' = direct ACT Ln
    (no DVE) — lets ACT absorb early chunks while DVE+sem latency warms.
"""

import numpy as np

import concourse.bass as bass
import concourse.tile as tile
from concourse import bacc
from concourse import mybir
from concourse import bass_utils

AF = mybir.ActivationFunctionType
ALU = mybir.AluOpType
AX = mybir.AxisListType
FP = mybir.dt.float32
F16 = mybir.dt.float16

B, T, K = 64, 4000, 100
WIN, OFFSET_D, TRUNC, SIGMA = 60, 40, 21, 9
EPS = 1e-8
NCORES = 8
BLOC = B // NCORES
AWC = 2 * WIN + BLOC            # auxW cols inside packed aux
APC = AWC + WIN                 # packed aux total cols
F16_BELOW_ONE = np.float16(1.0 - 2.0 ** -11)

# chunk plan: fractions of ftot (/32), tapering down; modes D=direct ACT,
# P=DVE-paired.  Direct first chunks keep ACT busy while the DVE chain
# and first sem latency warm up.
CHUNK_PLAN = [(8, "D"), (8, "P"), (8, "P"), (4, "P"), (2, "P"), (1, "P"),
              (1, "P")]


def _chunk_widths(ftot):
    assert ftot % 128 == 0
    s = ftot // 32
    ws = [f * s for f, _ in CHUNK_PLAN]
    ws[-1] += ftot - sum(ws)
    modes = [m for _, m in CHUNK_PLAN]
    assert sum(ws) == ftot and all(w > 0 and w % 4 == 0 for w in ws), ws
    return ws, modes


def _filt_np():
    half = TRUNC // 2
    x = np.arange(-half, half + 1, dtype=np.float32)
    g = np.exp(-0.5 * (x / SIGMA) ** 2).astype(np.float32)
    g = g / g.sum()
    f = np.zeros(WIN, np.float32)
    c = WIN // 2
    f[c - half:c + half + 1] = g
    return f


def _conv_matrix():
    f = _filt_np()
    pl = (WIN - 1) // 2
    idx = np.arange(WIN)
    u = idx[:, None] - idx[None, :] + pl
    M = np.where((u >= 0) & (u < WIN), f[np.clip(u, 0, WIN - 1)], 0.0)
    return M.astype(np.float32)


_NC_CACHE = {}


def _build_program(ftot):
    if ftot in _NC_CACHE:
        return _NC_CACHE[ftot]
    ws, modes = _chunk_widths(ftot)
    nch = len(ws)

    nc = bacc.Bacc("TRN2", debug=False)
    Vd = nc.dram_tensor("V", [128 * ftot], F16, kind="ExternalInput").ap()
    auxA = nc.dram_tensor("auxA", [WIN, APC], FP, kind="ExternalInput").ap()
    outd = nc.dram_tensor("out", [1, 1], FP, kind="ExternalOutput").ap()

    with tile.TileContext(nc) as tc:
        with tc.tile_pool(name="xin", bufs=1) as xin_pool, \
             tc.tile_pool(name="pair", bufs=1) as pair_pool, \
             tc.tile_pool(name="small", bufs=1) as small, \
             tc.tile_pool(name="psum", bufs=1, space="PSUM") as psum:

            # packed aux on the scalar HWDGE ring
            auxA_sb = small.tile([WIN, APC], FP)
            nc.scalar.dma_start(out=auxA_sb[:], in_=auxA)

            win_sl = auxA_sb[0:BLOC, 0:WIN]
            valid_sl = auxA_sb[0:BLOC, WIN:2 * WIN]
            I8_sl = auxA_sb[0:BLOC, 2 * WIN:AWC]
            M_sl = auxA_sb[0:WIN, AWC:APC]

            Call = small.tile([128, nch + 2], FP)
            nc.vector.memset(Call[:], 0.0)

            # bulk DMAs: chunk i is a contiguous 128*w block of the slab
            xtiles = []
            off = 0
            for i, w in enumerate(ws):
                xb = xin_pool.tile([128, w], F16, tag=f"xb{i}")
                nc.sync.dma_start(
                    out=xb[:],
                    in_=Vd[off:off + 128 * w].rearrange("(p f) -> p f", p=128))
                xtiles.append(xb)
                off += 128 * w

            # lnw early: warms the Ln table during the stream
            lnw = small.tile([BLOC, WIN], FP)
            nc.scalar.activation(out=lnw[:], in_=win_sl, func=AF.Ln,
                                 bias=1.0, scale=-1.0)

            # bulk chunks
            for i, w in enumerate(ws):
                xb = xtiles[i]
                if modes[i] == "D":
                    nc.scalar.activation(out=xb[:], in_=xb[:], func=AF.Ln,
                                         bias=1.0, scale=-1.0,
                                         accum_out=Call[:, i:i + 1])
                else:
                    h = w // 2
                    nc.vector.tensor_scalar(out=xb[:], in0=xb[:],
                                            scalar1=-64.0, scalar2=64.0,
                                            op0=ALU.mult, op1=ALU.add)
                    pr = pair_pool.tile([128, h], F16, tag=f"pr{i}")
                    nc.vector.tensor_tensor(out=pr[:], in0=xb[:, 0:h],
                                            in1=xb[:, h:w], op=ALU.mult)
                    nc.scalar.activation(out=pr[:], in_=pr[:], func=AF.Ln,
                                         scale=2.0 ** -12,
                                         accum_out=Call[:, i:i + 1])

            # window path (tiny; Tile schedules the DVE/PE ops into stream
            # slack, the final Ln lands after the chunk ACTs)
            lnwv = small.tile([BLOC, WIN], FP)
            nc.vector.tensor_tensor(out=lnwv[:], in0=lnw[:], in1=valid_sl,
                                    op=ALU.mult)
            ecol = small.tile([BLOC, 1], FP)
            nc.vector.tensor_reduce(out=ecol[:], in_=lnwv[:], axis=AX.X,
                                    op=ALU.add)
            nc.vector.tensor_scalar_mul(Call[0:BLOC, nch:nch + 1], ecol[:],
                                        -1.0)
            winv = small.tile([BLOC, WIN], FP)
            nc.vector.tensor_tensor(out=winv[:], in0=win_sl, in1=valid_sl,
                                    op=ALU.mult)
            wvt_ps = psum.tile([WIN, BLOC], FP)
            nc.tensor.matmul(out=wvt_ps[:], lhsT=winv[:], rhs=I8_sl,
                             start=True, stop=True)
            wvt = small.tile([WIN, BLOC], FP)
            nc.vector.tensor_copy(out=wvt[:], in_=wvt_ps[:])
            sm_ps = psum.tile([BLOC, WIN], FP)
            nc.tensor.matmul(out=sm_ps[:], lhsT=wvt[:], rhs=M_sl,
                             start=True, stop=True)
            smc = small.tile([BLOC, WIN], FP)
            nc.vector.tensor_scalar(out=smc[:], in0=sm_ps[:],
                                    scalar1=EPS, scalar2=1.0,
                                    op0=ALU.max, op1=ALU.min)
            smv = small.tile([BLOC, WIN], FP)
            nc.vector.tensor_tensor(out=smv[:], in0=smc[:], in1=valid_sl,
                                    op=ALU.mult)
            mx = small.tile([BLOC, 1], FP)
            nc.vector.tensor_reduce(out=mx[:], in_=smv[:], axis=AX.X,
                                    op=ALU.max)
            nc.scalar.activation(out=Call[0:BLOC, nch + 1:nch + 2],
                                 in_=mx[:], func=AF.Ln)

            # final: loss = -(sum of all columns)
            ones = small.tile([128, 1], FP)
            nc.vector.memset(ones[:], 1.0)
            tot_ps = psum.tile([1, nch + 2], FP)
            nc.tensor.matmul(out=tot_ps[:], lhsT=ones[:], rhs=Call[:],
                             start=True, stop=True)
            tot = small.tile([1, 1], FP)
            nc.vector.tensor_reduce(out=tot[:], in_=tot_ps[:], axis=AX.X,
                                    op=ALU.add, negate=True)
            nc.sync.dma_start(out=outd, in_=tot[:])

    nc.compile()
    _NC_CACHE[ftot] = nc
    return nc


def _make_in_maps(X, lengths, tgt, w_end):
    X = np.ascontiguousarray(np.asarray(X, dtype=np.float32))
    lengths = np.asarray(lengths, dtype=np.int64)
    tgt = np.asarray(tgt, dtype=np.int64)
    w_end = np.asarray(w_end, dtype=np.int64)

    tau_s = np.maximum(0, w_end + OFFSET_D - WIN)
    tau_e = np.minimum(tau_s + WIN, lengths)
    Lw = tau_e - tau_s

    # RN cast to f16; x that rounds to 1.0 clamps to 1-2^-11
    Xh = X.astype(np.float16)
    np.minimum(Xh, F16_BELOW_ONE, out=Xh)

    R = int(lengths.sum())
    ftot = -(-R * K // (NCORES * 128))
    ftot += (-ftot) % 128
    epc = 128 * ftot
    Vall = np.zeros(NCORES * epc, np.float16)
    pos = 0
    for b in range(B):
        n = int(lengths[b]) * K
        Vall[pos:pos + n] = Xh[b, :lengths[b], :].reshape(-1)
        pos += n

    idx = tau_s[:, None] + np.arange(WIN)[None, :]
    win = X[np.arange(B)[:, None], idx, tgt[:, None]]
    valid = (np.arange(WIN)[None, :] < Lw[:, None]).astype(np.float32)
    I8 = np.eye(BLOC, dtype=np.float32)
    Mmat = _conv_matrix()

    in_maps = []
    for cr in range(NCORES):
        bs = slice(cr * BLOC, (cr + 1) * BLOC)
        auxA = np.zeros((WIN, APC), np.float32)
        auxA[0:BLOC, 0:WIN] = win[bs]
        auxA[0:BLOC, WIN:2 * WIN] = valid[bs]
        auxA[0:BLOC, 2 * WIN:AWC] = I8
        auxA[0:WIN, AWC:APC] = Mmat
        in_maps.append({
            "V": Vall[cr * epc:(cr + 1) * epc],
            "auxA": auxA,
        })
    return ftot, in_maps


def kernel(X, lengths, tgt, w_end):
    ftot, in_maps = _make_in_maps(X, lengths, tgt, w_end)
    nc = _build_program(ftot)
    res = bass_utils.run_bass_kernel_spmd(
        nc, in_maps, core_ids=list(range(NCORES)))
    total = np.float32(0.0)
    for c in range(NCORES):
        total += np.float32(res.results[c]["out"][0, 0])
    return np.array(total, dtype=np.float32)
